# revision 36
# baseline (speedup 1.0000x reference)
"""MoE gate (DeepSeek-style top-8 router) on 8 Trainium2 cores.

Full-input contract: kernel(x, gate_w) -> (topk_w, topk_idx, aux_loss).

Strategy (data-parallel over tokens, per the sharding hint):
  - Host: transpose each 1024-token shard of x to [7168, 1024] so the
    contraction dim lands on SBUF partitions; fold the 2.5 route scale
    into a replicated wT = (2.5*gate_w).T [7168, 256].
  - Device (per core): logits[t,e] accumulated in PSUM over 56 K-chunks.
    Two matmul modes:
      fp32   -- true-fp32 matmuls (4 PE passes, 4 cyc/row)
      bf16x3 -- split each fp32 operand into bf16 hi + bf16 lo and compute
                xh@wh + xh@wl + xl@wh (3 bf16 passes, 3 cyc/row, ~1.33x
                faster; drops only the ~2^-18 lo*lo term)
    Softmax numerator via ACT Exp (bias = -rowmax, accum_out = denominator),
    top-8 of the *logits* via the DVE max/max_index instructions (descending
    values, lowest-index-first ties -- identical to jax.lax.top_k),
    renormalized top-8 weights, and a running [128,256] score accumulator
    reduced across partitions by a final ones-matmul into the per-expert
    score-sum partial.
  - Host: concat w/idx shards; p = sum(partials)/8192, f from a bincount of
    the indices, aux = 256 * sum(f*p).
"""

import os
import sys

import numpy as np

for _p in ("/opt/trn_rl_repo", "/root/.axon_site/_ro/trn_rl_repo"):
    if os.path.isdir(_p) and _p not in sys.path:
        sys.path.append(_p)

import ml_dtypes  # noqa: E402

import concourse.bass as bass  # noqa: E402
import concourse.tile as tile  # noqa: E402
from concourse import bacc, mybir  # noqa: E402
from concourse.bass_utils import run_bass_kernel_spmd  # noqa: E402

DIM = 7168
N_EXPERTS = 256
TOP_K = 8
ROUTE_SCALE = 2.5
N_TOKENS = 8192
N_CORES = 8
TPC = N_TOKENS // N_CORES          # tokens per core = 1024
KC = DIM // 128                    # contraction chunks = 56
G = 256                            # tokens per x-DMA group
NG = TPC // G                      # groups per core = 4
NB = G // 128                      # 128-token blocks per group = 2
NBLK = TPC // 128                  # blocks per core = 8

F32 = mybir.dt.float32
BF16 = mybir.dt.bfloat16
U32 = mybir.dt.uint32
BF16_NP = ml_dtypes.bfloat16

MODE = os.environ.get("MOE_GATE_MODE", "bf16x3")  # "fp32" | "bf16x3"


def _emit_epilogue(nc, tc, pl, spool, k8p, i8_d, lg_d, blk):
    """Top-8 routing for one 128-token block whose logits sit in PSUM tile
    `pl`.  Selection (order + tie semantics identical to jax.lax.top_k)
    happens here; the full logits ship to DRAM and the host does the 8-wide
    softmax normalization and the aux-loss reduction from them.  No ACT
    instructions remain in the program, which also drops the 1.3us
    ACT_TABLE_LOAD from the preamble."""
    sl = spool.tile([128, N_EXPERTS], F32, tag="sl")
    nc.vector.tensor_copy(sl[:], pl[:])

    v8 = k8p.tile([128, TOP_K], F32, tag="v8")
    nc.vector.max(v8[:], sl[:])
    i8 = k8p.tile([128, TOP_K], U32, tag="i8")
    nc.vector.max_index(i8[:], v8[:], sl[:])

    nc.sync.dma_start(i8_d[blk * 128:(blk + 1) * 128, :], i8[:])
    nc.sync.dma_start(lg_d[blk * 128:(blk + 1) * 128, :], sl[:])


def build_nc(mode=MODE):
    """Build + compile the per-core Bass program (SPMD: same program on all
    8 cores, different input data)."""
    nc = bacc.Bacc("TRN2", target_bir_lowering=False, debug=False,
                   num_devices=N_CORES)

    i8_d = nc.dram_tensor("i8", [TPC, TOP_K], U32, kind="ExternalOutput")
    lg_d = nc.dram_tensor("lg", [TPC, N_EXPERTS], F32, kind="ExternalOutput")

    # Inputs are host-packed to put the partition dim first so every DMA
    # moves multi-KB contiguous runs per partition line (512B-1KB lines made
    # the transfer descriptor-bound: ~72K descriptors @ ~85ns/queue starved
    # the PE).  x_packed[p, g, k, t] = x.T[k*128+p, g*G+t].
    if mode == "fp32":
        x_drams = [nc.dram_tensor("xp", [128, NG, KC, G], F32,
                                  kind="ExternalInput")]
        w_drams = [nc.dram_tensor("wp", [128, KC, N_EXPERTS], F32,
                                  kind="ExternalInput")]
        dt = F32
    elif mode == "bf16x3":
        x_drams = [nc.dram_tensor(n, [128, NG, KC, G], BF16,
                                  kind="ExternalInput") for n in ("xph", "xpl")]
        w_drams = [nc.dram_tensor(n, [128, KC, N_EXPERTS], BF16,
                                  kind="ExternalInput") for n in ("wph", "wpl")]
        dt = BF16
    else:
        raise ValueError(mode)

    with tile.TileContext(nc) as tc:
        with (
            tc.tile_pool(name="wpool", bufs=1) as wpool,
            tc.tile_pool(name="xpool", bufs=2) as xpool,
            tc.tile_pool(name="spool", bufs=3) as spool,
            tc.tile_pool(name="acc", bufs=1) as accp,
            tc.tile_pool(name="tiny", bufs=12) as tiny,
            tc.tile_pool(name="k8", bufs=4) as k8p,
            tc.tile_pool(name="psl", bufs=4, space=bass.MemorySpace.PSUM) as psl,
        ):
            # sub-DMAs of 7 k-chunks each: first matmuls start after ~2 of
            # them, while keeping lines contiguous (7 chunks x E x dt per
            # partition per transfer).  Descriptor generation on the SP
            # sequencer is ~0.9us per dma_start, so the issue ORDER matters:
            # interleave the group-0 x chunks with the w chunks so the k=0
            # matmuls have data after ~4 descriptor-gens instead of ~18.
            SUB = 7
            w_sbs = []
            for i, wd in enumerate(w_drams):
                wsb = wpool.tile([128, KC, N_EXPERTS], dt, tag=f"w{i}")
                w_sbs.append(wsb)
            xg0s = [xpool.tile([128, KC, G], dt, tag=f"x{i}", name=f"xg{i}")
                    for i in range(len(x_drams))]
            for k0 in range(0, KC, SUB):
                for wsb, wd in zip(w_sbs, w_drams):
                    nc.sync.dma_start(wsb[:, k0:k0 + SUB, :],
                                      wd[:, k0:k0 + SUB, :])
                for xg, xd in zip(xg0s, x_drams):
                    nc.sync.dma_start(xg[:, k0:k0 + SUB, :],
                                      xd[:, 0, k0:k0 + SUB, :])

            # dummy matmuls on a zeroed scratch tile: keep the PE busy while
            # the first data DMAs land so the HAM clock-gate is already at
            # 8/8 (2.4 GHz) when the real stream begins
            warm = accp.tile([128, N_EXPERTS], dt, name="warm")
            nc.vector.memset(warm[:], 0.0)
            with tc.tile_pool(name="pswarm", bufs=1,
                              space=bass.MemorySpace.PSUM) as pswarm:
                wps = pswarm.tile([128, N_EXPERTS], F32)
                nwarm = 16 if mode == "bf16x3" else 8
                for i in range(nwarm):
                    nc.tensor.matmul(wps[:], warm[:, :128], warm[:],
                                     start=(i == 0), stop=(i == nwarm - 1))



            for g in range(NG):
                if g == 0:
                    xgs = xg0s
                else:
                    # prefetched a whole group ahead -- coarser sub-DMAs
                    # (14 chunks, 7KB lines) halve the SP descriptor-gen load
                    xgs = [xpool.tile([128, KC, G], dt, tag=f"x{i}",
                                      name=f"xg{i}")
                           for i in range(len(x_drams))]
                    for k0 in range(0, KC, 2 * SUB):
                        for xg, xd in zip(xgs, x_drams):
                            nc.sync.dma_start(xg[:, k0:k0 + 2 * SUB, :],
                                              xd[:, g, k0:k0 + 2 * SUB, :])

                for b in range(NB):
                    blk = g * NB + b
                    ts = slice(b * 128, (b + 1) * 128)
                    pl = psl.tile([128, N_EXPERTS], F32)
                    for k in range(KC):
                        if mode == "fp32":
                            nc.tensor.matmul(pl[:], xgs[0][:, k, ts],
                                             w_sbs[0][:, k, :],
                                             start=(k == 0),
                                             stop=(k == KC - 1))
                        else:
                            # xh@wh + xh@wl + xl@wh (lo*lo dropped)
                            nc.tensor.matmul(pl[:], xgs[0][:, k, ts],
                                             w_sbs[0][:, k, :],
                                             start=(k == 0), stop=False)
                            nc.tensor.matmul(pl[:], xgs[0][:, k, ts],
                                             w_sbs[1][:, k, :],
                                             start=False, stop=False)
                            nc.tensor.matmul(pl[:], xgs[1][:, k, ts],
                                             w_sbs[0][:, k, :],
                                             start=False, stop=(k == KC - 1))

                    _emit_epilogue(nc, tc, pl, spool, k8p, i8_d, lg_d, blk)

    nc.compile()
    return nc


_NC = {}


def _get_nc(mode=MODE):
    if mode not in _NC:
        _NC[mode] = build_nc(mode)
    return _NC[mode]


def _pack_x(shard):
    """[TPC, DIM] -> [128, NG, KC, G] with x_packed[p,g,k,t] = shard[g*G+t, k*128+p]."""
    return np.ascontiguousarray(
        shard.reshape(NG, G, KC, 128).transpose(3, 0, 2, 1))


def _pack_w(wt):
    """[DIM, E] -> [128, KC, E]."""
    return np.ascontiguousarray(
        wt.reshape(KC, 128, N_EXPERTS).transpose(1, 0, 2))


def make_in_maps(x, gate_w, mode=MODE):
    x = np.asarray(x, dtype=np.float32)
    gate_w = np.asarray(gate_w, dtype=np.float32)
    assert x.shape == (N_TOKENS, DIM), x.shape
    assert gate_w.shape == (N_EXPERTS, DIM), gate_w.shape
    wt = (gate_w * np.float32(ROUTE_SCALE)).T  # [DIM, E]
    in_maps = []
    if mode == "fp32":
        wp = _pack_w(wt)
        for c in range(N_CORES):
            xp = _pack_x(x[c * TPC:(c + 1) * TPC, :])
            in_maps.append({"xp": xp, "wp": wp})
    else:
        wp32 = _pack_w(wt)
        wh = wp32.astype(BF16_NP)
        wl = (wp32 - wh.astype(np.float32)).astype(BF16_NP)
        for c in range(N_CORES):
            xp32 = _pack_x(x[c * TPC:(c + 1) * TPC, :])
            xh = xp32.astype(BF16_NP)
            xlo = (xp32 - xh.astype(np.float32)).astype(BF16_NP)
            in_maps.append({"xph": xh, "xpl": xlo, "wph": wh, "wpl": wl})
    return in_maps


def combine_results(results):
    topk_idx = np.concatenate([r["i8"] for r in results], axis=0).astype(np.int32)
    lg = np.concatenate([r["lg"] for r in results], axis=0)
    # renormalized top-8 weights: softmax over the 8 selected logits
    # (device-selected indices; values gathered from the device logits)
    v8 = np.take_along_axis(lg, topk_idx, axis=1)
    e8 = np.exp(v8 - v8[:, :1])          # v8 is descending; col 0 is the max
    topk_w = (e8 / e8.sum(axis=1, keepdims=True)).astype(np.float32)
    # p = mean softmax score per expert, for the aux loss
    m = lg.max(axis=1, keepdims=True)
    e = np.exp(lg - m)
    p = (e / e.sum(axis=1, keepdims=True)).mean(axis=0, dtype=np.float32)
    f = (np.bincount(topk_idx.ravel(), minlength=N_EXPERTS)
         .astype(np.float32) / np.float32(N_TOKENS))
    aux_loss = np.float32(np.sum(f * p, dtype=np.float32) * np.float32(N_EXPERTS))
    return topk_w, topk_idx, aux_loss


def run_raw(x, gate_w):
    nc = _get_nc()
    in_maps = make_in_maps(x, gate_w)
    return run_bass_kernel_spmd(nc, in_maps, list(range(N_CORES))).results


def _run_in_subprocess(x, gate_w):
    """Occasional transient NRT_EXEC_UNIT_UNRECOVERABLE wedges the in-process
    PJRT client; a fresh process re-opens the recovered device."""
    import subprocess
    import tempfile

    d = tempfile.mkdtemp(prefix="moegate_")
    in_path = os.path.join(d, "in.npz")
    out_path = os.path.join(d, "out.npz")
    np.savez(in_path, x=np.asarray(x, np.float32),
             gate_w=np.asarray(gate_w, np.float32))
    code = (
        "import sys, numpy as np\n"
        f"sys.path.insert(0, {os.path.dirname(os.path.abspath(__file__))!r})\n"
        "import kernel\n"
        f"d = np.load({in_path!r})\n"
        "r = kernel.run_raw(d['x'], d['gate_w'])\n"
        f"np.savez({out_path!r}, "
        "**{f'{k}_{i}': v for i, m in enumerate(r) for k, v in m.items()})\n"
    )
    subprocess.run([sys.executable, "-c", code], check=True, timeout=1800)
    z = np.load(out_path)
    return [{k: z[f"{k}_{i}"] for k in ("i8", "lg")}
            for i in range(N_CORES)]


def kernel(x, gate_w):
    try:
        res = run_raw(x, gate_w)
    except Exception:
        res = _run_in_subprocess(x, gate_w)
    return combine_results(res)


# revision 41
# speedup vs baseline: 1.0446x; 1.0446x over previous
"""MoE gate (DeepSeek-style top-8 router) on 8 Trainium2 cores.

Full-input contract: kernel(x, gate_w) -> (topk_w, topk_idx, aux_loss).

Strategy (data-parallel over tokens, per the sharding hint):
  - Host: transpose each 1024-token shard of x to [7168, 1024] so the
    contraction dim lands on SBUF partitions; fold the 2.5 route scale
    into a replicated wT = (2.5*gate_w).T [7168, 256].
  - Device (per core): logits[t,e] accumulated in PSUM over 56 K-chunks.
    Two matmul modes:
      fp32   -- true-fp32 matmuls (4 PE passes, 4 cyc/row)
      bf16x3 -- split each fp32 operand into bf16 hi + bf16 lo and compute
                xh@wh + xh@wl + xl@wh (3 bf16 passes, 3 cyc/row, ~1.33x
                faster; drops only the ~2^-18 lo*lo term)
    Softmax numerator via ACT Exp (bias = -rowmax, accum_out = denominator),
    top-8 of the *logits* via the DVE max/max_index instructions (descending
    values, lowest-index-first ties -- identical to jax.lax.top_k),
    renormalized top-8 weights, and a running [128,256] score accumulator
    reduced across partitions by a final ones-matmul into the per-expert
    score-sum partial.
  - Host: concat w/idx shards; p = sum(partials)/8192, f from a bincount of
    the indices, aux = 256 * sum(f*p).
"""

import os
import sys

import numpy as np

for _p in ("/opt/trn_rl_repo", "/root/.axon_site/_ro/trn_rl_repo"):
    if os.path.isdir(_p) and _p not in sys.path:
        sys.path.append(_p)

import ml_dtypes  # noqa: E402

import concourse.bass as bass  # noqa: E402
import concourse.tile as tile  # noqa: E402
from concourse import bacc, mybir  # noqa: E402
from concourse.bass_utils import run_bass_kernel_spmd  # noqa: E402

DIM = 7168
N_EXPERTS = 256
TOP_K = 8
ROUTE_SCALE = 2.5
N_TOKENS = 8192
N_CORES = 8
TPC = N_TOKENS // N_CORES          # tokens per core = 1024
KC = DIM // 128                    # contraction chunks = 56
G = 256                            # tokens per x-DMA group
NG = TPC // G                      # groups per core = 4
NB = G // 128                      # 128-token blocks per group = 2
NBLK = TPC // 128                  # blocks per core = 8

F32 = mybir.dt.float32
BF16 = mybir.dt.bfloat16
U32 = mybir.dt.uint32
BF16_NP = ml_dtypes.bfloat16

MODE = os.environ.get("MOE_GATE_MODE", "bf16x3")  # "fp32" | "bf16x3"


def _emit_epilogue(nc, tc, pl, spool, k8p, i8_d, lg_d, blk, wide):
    """Top-8 routing for one 128-token block whose logits sit in PSUM tile
    `pl`.  Selection (order + tie semantics identical to jax.lax.top_k)
    happens here; the full logits ship to DRAM and the host does the 8-wide
    softmax normalization and the aux-loss reduction from them.  No ACT
    instructions remain in the program.  In `wide` (fused-rhs) mode the
    xh@wl partial sits in the upper 256 columns of the bank and is folded
    in here."""
    sl = spool.tile([128, N_EXPERTS], F32, tag="sl")
    if wide:
        t0 = spool.tile([128, N_EXPERTS], F32, tag="t0")
        nc.vector.tensor_copy(t0[:], pl[:, N_EXPERTS:])
        nc.vector.tensor_add(sl[:], t0[:], pl[:, :N_EXPERTS])
    else:
        nc.vector.tensor_copy(sl[:], pl[:])

    v8 = k8p.tile([128, TOP_K], F32, tag="v8")
    nc.vector.max(v8[:], sl[:])
    i8 = k8p.tile([128, TOP_K], U32, tag="i8")
    nc.vector.max_index(i8[:], v8[:], sl[:])

    nc.sync.dma_start(i8_d[blk * 128:(blk + 1) * 128, :], i8[:])
    nc.sync.dma_start(lg_d[blk * 128:(blk + 1) * 128, :], sl[:])


def build_nc(mode=MODE):
    """Build + compile the per-core Bass program (SPMD: same program on all
    8 cores, different input data)."""
    nc = bacc.Bacc("TRN2", target_bir_lowering=False, debug=False,
                   num_devices=N_CORES)

    i8_d = nc.dram_tensor("i8", [TPC, TOP_K], U32, kind="ExternalOutput")
    lg_d = nc.dram_tensor("lg", [TPC, N_EXPERTS], F32, kind="ExternalOutput")

    # Inputs are host-packed to put the partition dim first so every DMA
    # moves multi-KB contiguous runs per partition line (512B-1KB lines made
    # the transfer descriptor-bound: ~72K descriptors @ ~85ns/queue starved
    # the PE).  x_packed[p, g, k, t] = x.T[k*128+p, g*G+t].
    if mode == "fp32":
        x_drams = [nc.dram_tensor("xp", [128, NG, KC, G], F32,
                                  kind="ExternalInput")]
        w_drams = [nc.dram_tensor("wp", [128, KC, N_EXPERTS], F32,
                                  kind="ExternalInput")]
        dt = F32
    elif mode == "bf16x3":
        x_drams = [nc.dram_tensor(n, [128, NG, KC, G], BF16,
                                  kind="ExternalInput") for n in ("xph", "xpl")]
        # wh and wl fused side by side: xh@[wh|wl] is ONE N=512 matmul
        w_drams = [nc.dram_tensor("wpk", [128, KC, 2 * N_EXPERTS], BF16,
                                  kind="ExternalInput")]
        dt = BF16
    else:
        raise ValueError(mode)

    with tile.TileContext(nc) as tc:
        with (
            tc.tile_pool(name="wpool", bufs=1) as wpool,
            tc.tile_pool(name="xpool", bufs=2) as xpool,
            tc.tile_pool(name="spool", bufs=3) as spool,
            tc.tile_pool(name="acc", bufs=1) as accp,
            tc.tile_pool(name="tiny", bufs=12) as tiny,
            tc.tile_pool(name="k8", bufs=4) as k8p,
            tc.tile_pool(name="psl", bufs=4, space=bass.MemorySpace.PSUM) as psl,
        ):
            # sub-DMAs of 7 k-chunks each: first matmuls start after ~2 of
            # them, while keeping lines contiguous (7 chunks x E x dt per
            # partition per transfer).  Descriptor generation on the SP
            # sequencer is ~0.9us per dma_start, so the issue ORDER matters:
            # interleave the group-0 x chunks with the w chunks so the k=0
            # matmuls have data after ~4 descriptor-gens instead of ~18.
            SUB = 7
            wwid = 2 * N_EXPERTS if mode == "bf16x3" else N_EXPERTS
            plw = wwid
            w_sbs = []
            for i, wd in enumerate(w_drams):
                wsb = wpool.tile([128, KC, wwid], dt, tag=f"w{i}")
                w_sbs.append(wsb)
            xg0s = [xpool.tile([128, KC, G], dt, tag=f"x{i}", name=f"xg{i}")
                    for i in range(len(x_drams))]
            for k0 in range(0, KC, SUB):
                for wsb, wd in zip(w_sbs, w_drams):
                    nc.sync.dma_start(wsb[:, k0:k0 + SUB, :],
                                      wd[:, k0:k0 + SUB, :])
                for xg, xd in zip(xg0s, x_drams):
                    nc.sync.dma_start(xg[:, k0:k0 + SUB, :],
                                      xd[:, 0, k0:k0 + SUB, :])

            # dummy matmuls on a zeroed scratch tile: keep the PE busy while
            # the first data DMAs land so the HAM clock-gate is already at
            # 8/8 (2.4 GHz) when the real stream begins
            warm = accp.tile([128, N_EXPERTS], dt, name="warm")
            nc.vector.memset(warm[:], 0.0)
            with tc.tile_pool(name="pswarm", bufs=1,
                              space=bass.MemorySpace.PSUM) as pswarm:
                wps = pswarm.tile([128, N_EXPERTS], F32)
                nwarm = 16 if mode == "bf16x3" else 8
                for i in range(nwarm):
                    nc.tensor.matmul(wps[:], warm[:, :128], warm[:],
                                     start=(i == 0), stop=(i == nwarm - 1))



            for g in range(NG):
                if g == 0:
                    xgs = xg0s
                else:
                    # prefetched a whole group ahead -- coarser sub-DMAs
                    # (14 chunks, 7KB lines) halve the SP descriptor-gen load
                    xgs = [xpool.tile([128, KC, G], dt, tag=f"x{i}",
                                      name=f"xg{i}")
                           for i in range(len(x_drams))]
                    for k0 in range(0, KC, 2 * SUB):
                        for xg, xd in zip(xgs, x_drams):
                            nc.sync.dma_start(xg[:, k0:k0 + 2 * SUB, :],
                                              xd[:, g, k0:k0 + 2 * SUB, :])

                for b in range(NB):
                    blk = g * NB + b
                    ts = slice(b * 128, (b + 1) * 128)
                    pl = psl.tile([128, plw], F32)
                    for k in range(KC):
                        if mode == "fp32":
                            nc.tensor.matmul(pl[:], xgs[0][:, k, ts],
                                             w_sbs[0][:, k, :],
                                             start=(k == 0),
                                             stop=(k == KC - 1))
                        else:
                            # xh@[wh|wl] (N=512) + xl@wh into the low half;
                            # the two bank halves are folded in the epilogue
                            nc.tensor.matmul(pl[:], xgs[0][:, k, ts],
                                             w_sbs[0][:, k, :],
                                             start=(k == 0), stop=False)
                            nc.tensor.matmul(pl[:, :N_EXPERTS],
                                             xgs[1][:, k, ts],
                                             w_sbs[0][:, k, :N_EXPERTS],
                                             start=False, stop=(k == KC - 1))

                    _emit_epilogue(nc, tc, pl, spool, k8p, i8_d, lg_d, blk,
                                   wide=(mode == "bf16x3"))

    nc.compile()
    return nc


_NC = {}


def _get_nc(mode=MODE):
    if mode not in _NC:
        _NC[mode] = build_nc(mode)
    return _NC[mode]


def _pack_x(shard):
    """[TPC, DIM] -> [128, NG, KC, G] with x_packed[p,g,k,t] = shard[g*G+t, k*128+p]."""
    return np.ascontiguousarray(
        shard.reshape(NG, G, KC, 128).transpose(3, 0, 2, 1))


def _pack_w(wt):
    """[DIM, E] -> [128, KC, E]."""
    return np.ascontiguousarray(
        wt.reshape(KC, 128, N_EXPERTS).transpose(1, 0, 2))


def make_in_maps(x, gate_w, mode=MODE):
    x = np.asarray(x, dtype=np.float32)
    gate_w = np.asarray(gate_w, dtype=np.float32)
    assert x.shape == (N_TOKENS, DIM), x.shape
    assert gate_w.shape == (N_EXPERTS, DIM), gate_w.shape
    wt = (gate_w * np.float32(ROUTE_SCALE)).T  # [DIM, E]
    in_maps = []
    if mode == "fp32":
        wp = _pack_w(wt)
        for c in range(N_CORES):
            xp = _pack_x(x[c * TPC:(c + 1) * TPC, :])
            in_maps.append({"xp": xp, "wp": wp})
    else:
        wp32 = _pack_w(wt)
        wh = wp32.astype(BF16_NP)
        wl = (wp32 - wh.astype(np.float32)).astype(BF16_NP)
        wpk = np.ascontiguousarray(np.concatenate([wh, wl], axis=2))
        for c in range(N_CORES):
            xp32 = _pack_x(x[c * TPC:(c + 1) * TPC, :])
            xh = xp32.astype(BF16_NP)
            xlo = (xp32 - xh.astype(np.float32)).astype(BF16_NP)
            in_maps.append({"xph": xh, "xpl": xlo, "wpk": wpk})
    return in_maps


def combine_results(results):
    topk_idx = np.concatenate([r["i8"] for r in results], axis=0).astype(np.int32)
    lg = np.concatenate([r["lg"] for r in results], axis=0)
    # renormalized top-8 weights: softmax over the 8 selected logits
    # (device-selected indices; values gathered from the device logits)
    v8 = np.take_along_axis(lg, topk_idx, axis=1)
    e8 = np.exp(v8 - v8[:, :1])          # v8 is descending; col 0 is the max
    topk_w = (e8 / e8.sum(axis=1, keepdims=True)).astype(np.float32)
    # p = mean softmax score per expert, for the aux loss
    m = lg.max(axis=1, keepdims=True)
    e = np.exp(lg - m)
    p = (e / e.sum(axis=1, keepdims=True)).mean(axis=0, dtype=np.float32)
    f = (np.bincount(topk_idx.ravel(), minlength=N_EXPERTS)
         .astype(np.float32) / np.float32(N_TOKENS))
    aux_loss = np.float32(np.sum(f * p, dtype=np.float32) * np.float32(N_EXPERTS))
    return topk_w, topk_idx, aux_loss


def run_raw(x, gate_w):
    nc = _get_nc()
    in_maps = make_in_maps(x, gate_w)
    return run_bass_kernel_spmd(nc, in_maps, list(range(N_CORES))).results


def _run_in_subprocess(x, gate_w):
    """Occasional transient NRT_EXEC_UNIT_UNRECOVERABLE wedges the in-process
    PJRT client; a fresh process re-opens the recovered device."""
    import subprocess
    import tempfile

    d = tempfile.mkdtemp(prefix="moegate_")
    in_path = os.path.join(d, "in.npz")
    out_path = os.path.join(d, "out.npz")
    np.savez(in_path, x=np.asarray(x, np.float32),
             gate_w=np.asarray(gate_w, np.float32))
    code = (
        "import sys, numpy as np\n"
        f"sys.path.insert(0, {os.path.dirname(os.path.abspath(__file__))!r})\n"
        "import kernel\n"
        f"d = np.load({in_path!r})\n"
        "r = kernel.run_raw(d['x'], d['gate_w'])\n"
        f"np.savez({out_path!r}, "
        "**{f'{k}_{i}': v for i, m in enumerate(r) for k, v in m.items()})\n"
    )
    subprocess.run([sys.executable, "-c", code], check=True, timeout=1800)
    z = np.load(out_path)
    return [{k: z[f"{k}_{i}"] for k in ("i8", "lg")}
            for i in range(N_CORES)]


def kernel(x, gate_w):
    try:
        res = run_raw(x, gate_w)
    except Exception:
        res = _run_in_subprocess(x, gate_w)
    return combine_results(res)
